# revision 1
# baseline (speedup 1.0000x reference)
"""Triangular pairwise channel product on 8 Trainium2 NeuronCores.

out[b,h,w,k] = x[b,h,w,i_k] * x[b,h,w,j_k]  for the C*(C-1)/2 pairs
(i<j) in row-major (np.triu_indices) order.

Sharding: pure data parallel over batch — core c takes x[2c:2c+2].
Per core the 2*64*64 = 8192 spatial positions map to 128 SBUF
partitions (b_loc*64+h) x 64 groups (w).  Block i of the output
(pairs (i, i+1..63)) is one tensor_tensor multiply whose first operand
is x[:, :, i] broadcast via a step-0 access pattern.

Measured HW facts driving the design (8 cores concurrent):
- HBM write cap ~363 GB/s/core; one HWDGE ring saturates it; dual
  rings / SWDGE / DMA-cast are all slower.  So y lives in DRAM as
  bf16 (33 MB vs 66 MB -> ~96 us stream) and kernel() upcasts on the
  host; rel err ~1e-3 vs the 2e-2 gate.
- fp32/bf16 broadcast multiply runs at 1 elem/cycle on DVE (0.96 GHz,
  stride-0 operand disqualifies the 2x_1p mode): 134 us variable +
  ~70 ns/op, so DVE (not DMA) is the wall once y is bf16.
- GpSimd tensor_mul measures ~2.5 ns/elem; a *decoupled* Pool
  pipeline (own channel range, own tiles, own ACT-ring stores) takes
  the widest blocks off DVE.  (A shared-tile split regressed badly:
  cross-engine sems + store gating.)
"""

import numpy as np

import concourse.bacc as bacc
import concourse.bass as bass
import concourse.mybir as mybir
import concourse.tile as tile
from concourse.bass_utils import run_bass_kernel_spmd

B, H, W, C = 16, 64, 64, 64
K = C * (C - 1) // 2  # 2016
N_CORES = 8
BP = B // N_CORES  # batch rows per core
P = BP * H         # 128 SBUF partitions
G_TOTAL = W        # position groups per partition
G_ITERS = [16, 16, 16, 8, 5, 3]
assert sum(G_ITERS) == G_TOTAL
FP = mybir.dt.float32
BF = mybir.dt.bfloat16

_row = [0]
for _i in range(C):
    _row.append(_row[-1] + C - 1 - _i)

_nc_cache = None


def build_bass(
    np_blocks: int = 10,
    g_iters: list[int] | None = None,
    y_bf16: bool = True,
    x_bf16: bool = False,
) -> bass.Bass:
    # Bacc (not plain Bass): its compile() pipeline runs
    # generate_event_semaphores, which splits multi-wait instructions to
    # satisfy the TRN2 1-wait-per-instruction codegen limit.
    nc = bacc.Bacc(
        "TRN2",
        target_bir_lowering=False,
        debug=False,
        num_devices=N_CORES,
    )
    if g_iters is None:
        g_iters = G_ITERS
    assert sum(g_iters) == G_TOTAL
    g0 = g_iters[0]
    ydt = BF if y_bf16 else FP
    xdt = BF if x_bf16 else FP
    kp = _row[np_blocks]  # Pool-owned channels [0, kp); DVE owns [kp, K)

    x = nc.dram_tensor("x", [P, G_TOTAL, C], xdt, kind="ExternalInput")
    y = nc.dram_tensor("y", [P, G_TOTAL, K], ydt, kind="ExternalOutput")

    with tile.TileContext(nc) as tc:
        with (
            tc.tile_pool(name="xin", bufs=1) as xpool,
            tc.tile_pool(name="dout", bufs=2) as dpool,
            tc.tile_pool(name="pout", bufs=2) as ppool,
        ):
            # Preload the input in two pieces: iteration 0's chunk on the
            # SP ring, the rest on the ACT ring.
            xt0 = xpool.tile([P, g0, C], xdt, tag="x0")
            nc.sync.dma_start(out=xt0[:], in_=x[:, 0:g0, :])
            xtr = xpool.tile([P, G_TOTAL - g0, C], xdt, tag="xr")
            nc.scalar.dma_start(out=xtr[:], in_=x[:, g0:, :])

            g_off = 0
            for it, Gi in enumerate(g_iters):
                if it == 0:
                    xg = xt0[:, :, :]
                else:
                    xg = xtr[:, g_off - g0 : g_off - g0 + Gi, :]

                if np_blocks > 0:
                    pt = ppool.tile([P, Gi, kp], ydt, tag="pt")
                    for i in range(np_blocks):
                        w = C - 1 - i
                        a = xg[:, :, i : i + 1].broadcast_to([P, Gi, w])
                        b = xg[:, :, i + 1 : C]
                        nc.gpsimd.tensor_mul(pt[:, :, _row[i] : _row[i] + w], a, b)
                    nc.scalar.dma_start(out=y[:, g_off : g_off + Gi, 0:kp], in_=pt[:])

                dt_ = dpool.tile([P, Gi, K - kp], ydt, tag="dt")
                for i in range(np_blocks, C - 1):
                    w = C - 1 - i
                    a = xg[:, :, i : i + 1].broadcast_to([P, Gi, w])
                    b = xg[:, :, i + 1 : C]
                    nc.vector.tensor_mul(
                        dt_[:, :, _row[i] - kp : _row[i] - kp + w], a, b
                    )
                nc.sync.dma_start(out=y[:, g_off : g_off + Gi, kp:K], in_=dt_[:])
                g_off += Gi

    nc.finalize()
    return nc


def build_dpack(g_iters: list[int] | None = None, bufs: int = 2) -> bass.Bass:
    """d-offset formulation: for d in 1..63 compute
    prod_d[p,g,c] = x[p,g,c] * x[p,g,c+d] for c in [0, 64-d).

    Unlike the block-i broadcast form, ALL three operands are step-1
    packed bf16, so DVE's 2x_1p perf mode applies (2 elem/cycle).
    Odd d would put the second operand at a 2-byte-aligned address
    (4B alignment required for 2x), so those ops read from x_odd, a
    one-channel-shifted copy of x.  Odd widths are padded up to even
    (the pad product is computed too, so every tile byte is written);
    padded widths sum to exactly 2048 channels.  The host undoes the
    (d,c)->(i,j) permutation with a numpy gather.
    """
    nc = bacc.Bacc(
        "TRN2",
        target_bir_lowering=False,
        debug=False,
        num_devices=N_CORES,
    )
    if g_iters is None:
        g_iters = G_ITERS
    assert sum(g_iters) == G_TOTAL
    g0 = g_iters[0]

    x = nc.dram_tensor("x", [P, G_TOTAL, C], BF, kind="ExternalInput")
    y = nc.dram_tensor("y", [P, G_TOTAL, KPAD], BF, kind="ExternalOutput")

    with tile.TileContext(nc) as tc:
        with (
            tc.tile_pool(name="xin", bufs=1) as xpool,
            tc.tile_pool(name="out", bufs=bufs) as opool,
        ):
            xt = xpool.tile([P, G_TOTAL, C], BF, tag="xt")
            nc.sync.dma_start(out=xt[:, 0:g0, :], in_=x[:, 0:g0, :])
            nc.scalar.dma_start(out=xt[:, g0:, :], in_=x[:, g0:, :])

            # One-channel-shifted copy (x_odd[c] = x[c+1]) so odd-d ops
            # stay 4B-aligned; last channel zeroed (read by padded ops).
            # Copies run on the otherwise-idle GpSimd engine, and each
            # iteration issues even-d ops (which only need xt) first, so
            # DVE never waits on the shift copy.
            xo = xpool.tile([P, G_TOTAL, C], BF, tag="xo")
            nc.gpsimd.memset(xo[:, :, C - 1 : C], 0.0)
            nc.vector.tensor_copy(xo[:, 0:g0, 0 : C - 1], xt[:, 0:g0, 1:C])
            nc.vector.tensor_copy(xo[:, g0:, 0 : C - 1], xt[:, g0:, 1:C])

            g_off = 0
            for it, Gi in enumerate(g_iters):
                gs = slice(g_off, g_off + Gi)
                ot = opool.tile([P, Gi, KPAD], BF, tag="ot")
                for d in sorted(range(1, C), key=lambda d: d % 2):
                    wp = _WPAD[d]
                    a = xt[:, gs, 0:wp]
                    if d % 2 == 0:
                        b = xt[:, gs, d : d + wp]
                    else:
                        b = xo[:, gs, d - 1 : d - 1 + wp]
                    nc.vector.tensor_mul(
                        ot[:, :, _QOFF[d] : _QOFF[d] + wp], a, b
                    )
                nc.sync.dma_start(out=y[:, gs, :], in_=ot[:])
                g_off += Gi

    nc.finalize()
    return nc


# Padded d-major layout: width of slot d (odd widths rounded up to even).
_WPAD = [0] + [(C - d) + ((C - d) % 2) for d in range(1, C)]
_QOFF = [0, 0]
for _d in range(1, C - 1):
    _QOFF.append(_QOFF[-1] + _WPAD[_d])
KPAD = _QOFF[C - 1] + _WPAD[C - 1]
assert KPAD == 2048, KPAD

# Host-side permutation: k (triu order) -> padded (d,c) position.
_II_, _JJ_ = np.triu_indices(C, k=1)
_PERM = np.asarray(_QOFF, dtype=np.int64)[_JJ_ - _II_] + _II_


def make_in_maps(x: np.ndarray, x_bf16: bool = False) -> list[dict[str, np.ndarray]]:
    x = np.ascontiguousarray(x, dtype=np.float32)
    if x_bf16:
        import ml_dtypes

        x = x.astype(ml_dtypes.bfloat16)
    return [
        {"x": x[c * BP : (c + 1) * BP].reshape(P, G_TOTAL, C)} for c in range(N_CORES)
    ]


def kernel(**inputs: np.ndarray) -> np.ndarray:
    global _nc_cache
    if _nc_cache is None:
        _nc_cache = build_dpack()
    res = run_bass_kernel_spmd(
        _nc_cache, make_in_maps(inputs["inputs"], x_bf16=True), list(range(N_CORES))
    ).results
    ypad = np.concatenate(
        [
            np.asarray(res[c]["y"]).reshape(BP, H, W, KPAD)
            for c in range(N_CORES)
        ],
        axis=0,
    )
    # Undo the padded (d,c) channel layout -> triu (i,j) order + upcast.
    return np.take(ypad, _PERM, axis=-1).astype(np.float32)

